# revision 2
# baseline (speedup 1.0000x reference)
"""ChebNet (2-layer ChebConv, K=3) on 8 Trainium2 NeuronCores.

Strategy
--------
Math: propagation commutes with the per-order weight matmul, so the two
ChebConv layers reduce to 4 sparse propagations on raw features plus tiny
dense matmuls:
    L1: out1 = x(W10-W12) + Tx1*W11 + 2*Tx2*W12 + b1,  Tx1 = L x, Tx2 = L Tx1
    h = relu(out1)
    L2: out2 = h(W20-W22) + U1*W21 + 2*U2*W22 + b2,    U1 = L h, U2 = L U1
where L[c,r] = sum over edges (r->c) of -dinv[r]*w*dinv[c]  (PyG ChebConv
normalization with lambda_max=2).

Sharding: each of the 8 cores owns edges with source in one of 4 contiguous
25024-row windows and dest in one of 2 halves (4 chunks x 2 halves). Each
core gathers source rows from its window (int16 dma_gather indices), forms
per-edge messages, and aggregates them into its half's destination tiles
with an is_equal-selector matmul on the tensor engine. Host reduces the 4
partial aggregates per half between launches (pure data movement + adds).

Device pipeline per pass: dma_gather (1024 rows/call) -> DVE builds
S[t,d] = norm[t] * (iota[d] == local_dest[t]) -> PE matmul accumulates
psum[d, :] += S^T @ messages -> psum copied out per 64-node dest tile.
"""
import numpy as np
from contextlib import ExitStack

import concourse.bass as bass
import concourse.bacc as bacc
import concourse.mybir as mybir
import concourse.tile as tile
from concourse.bass_utils import run_bass_kernel_spmd

# problem constants (hardcoded per harness contract)
N = 100000
E = 1600000
F_IN = 128
F_HID = 64
F_OUT = 40
K = 3

P = 128
D = 64                 # dest-tile width (nodes per psum tile)
NPAD = 100096          # padded node count: /128 = 782, /64 = 1564
NCHUNK = 4
CH = NPAD // NCHUNK    # 25024 source rows per chunk (< 32768 for int16 idx)
NHALF = 2
HALF = NPAD // NHALF   # 50048 dest rows per half
TS = HALF // D         # 782 dest tiles per half
NCORES = 8
NS = NPAD // NCORES    # 12512 nodes per core for dense epilogues
CALL_BLOCKS = 8        # 1024 gather rows per dma_gather (descriptor ring limit)

_DT = mybir.dt.float32


# ---------------------------------------------------------------------------
# host-side graph preprocessing
# ---------------------------------------------------------------------------

def _prep_graph(edge_index, edge_weight):
    """Partition edges into 8 (chunk, half) cores; build per-core slot arrays.

    Returns dict with per-core int16 gather indices (wrapped layout), meta
    (ld/norm) arrays, the shared block structure NB[slot->nblocks], call plan,
    and per-core tile-id permutations.
    """
    row = np.ascontiguousarray(edge_index[0]).astype(np.int64)
    col = np.ascontiguousarray(edge_index[1]).astype(np.int64)
    w = np.ascontiguousarray(edge_weight).astype(np.float32)

    deg = np.bincount(row, weights=w.astype(np.float64), minlength=N).astype(np.float32)
    dinv = np.where(deg > 0, 1.0 / np.sqrt(np.maximum(deg, 1e-30)), 0.0).astype(np.float32)
    norm = (-dinv[row] * w * dinv[col]).astype(np.float32)

    chunk = row // CH
    half = col // HALF
    core_of_edge = (half * NCHUNK + chunk).astype(np.int64)

    # per (core, tile) edge counts; tile id local to the half
    ltile = (col % HALF) // D
    ld = (col % HALF) % D

    cores = []
    nblocks_sorted = []
    for c in range(NCORES):
        sel = np.nonzero(core_of_edge == c)[0]
        # order edges by local tile for contiguous tile runs
        order = np.argsort(ltile[sel], kind="stable")
        sel = sel[order]
        t_of_e = ltile[sel]
        counts = np.bincount(t_of_e, minlength=TS)
        nb = np.maximum(1, -(-counts // P))  # ceil, min 1 block per tile
        # sort tiles by descending block count, stable by tile id
        perm = np.lexsort((np.arange(TS), -nb))
        cores.append(dict(sel=sel, counts=counts, nb=nb, perm=perm))
        nblocks_sorted.append(nb[perm])

    NB = np.max(np.stack(nblocks_sorted), axis=0)  # shared per-slot block count
    B_TOTAL = int(NB.sum())
    SLOTS = B_TOTAL * P

    # call plan: runs of <= CALL_BLOCKS blocks (identical for all cores)
    calls = []
    b = 0
    while b < B_TOTAL:
        n = min(CALL_BLOCKS, B_TOTAL - b)
        calls.append((b, n))
        b += n

    # per-core slot arrays; pad slots gather row 0 (zeroed by ld=-1 in S)
    idx16 = np.zeros((NCORES, SLOTS), np.int16)
    ld_f = np.full((NCORES, SLOTS), -1.0, np.float32)
    nrm_f = np.zeros((NCORES, SLOTS), np.float32)
    tile_ids = np.zeros((NCORES, TS), np.int64)

    slot_tile_start = np.concatenate([[0], np.cumsum(NB)]) * P  # per sorted slot
    for c in range(NCORES):
        st = cores[c]
        sel, counts, perm = st["sel"], st["counts"], st["perm"]
        tile_ids[c] = perm
        # edge offsets per tile in the tile-ordered edge list
        e_start = np.concatenate([[0], np.cumsum(counts)])
        ch_base = (c % NCHUNK) * CH
        for s in range(TS):
            t = perm[s]
            cnt = counts[t]
            if cnt == 0:
                continue
            eids = sel[e_start[t] : e_start[t] + cnt]
            base = slot_tile_start[s]
            idx16[c, base : base + cnt] = (row[eids] - ch_base).astype(np.int16)
            ld_f[c, base : base + cnt] = ld[eids].astype(np.float32)
            nrm_f[c, base : base + cnt] = norm[eids]

    # wrap idx to dma_gather layout [16, SLOTS/16] at [i%16, i//16], tile 8x
    ii = np.arange(SLOTS)
    idxw = np.zeros((NCORES, 16, SLOTS // 16), np.int16)
    idxw[:, ii % 16, ii // 16] = idx16
    idxw = np.tile(idxw, (1, 8, 1))  # [NCORES, 128, SLOTS/16]

    # meta [128, D + 2*B_TOTAL]: iota then per-block (ld, norm) column pairs.
    # slot i of call k maps to partition i%128, block (global) i//128.
    meta = np.zeros((NCORES, P, D + 2 * B_TOTAL), np.float32)
    meta[:, :, :D] = np.arange(D, dtype=np.float32)[None, None, :]
    ld_pb = ld_f.reshape(NCORES, B_TOTAL, P).transpose(0, 2, 1)   # [C, 128, B]
    nrm_pb = nrm_f.reshape(NCORES, B_TOTAL, P).transpose(0, 2, 1)
    meta[:, :, D::2] = ld_pb
    meta[:, :, D + 1 :: 2] = nrm_pb

    return dict(
        NB=NB, B_TOTAL=B_TOTAL, SLOTS=SLOTS, calls=calls,
        idxw=idxw, meta=meta, tile_ids=tile_ids, dinv=dinv,
    )


# ---------------------------------------------------------------------------
# device program builders
# ---------------------------------------------------------------------------

def _build_pass_program(F, NB, calls, B_TOTAL, SLOTS, dense=None):
    """One propagation pass: gather + selector-matmul aggregation.

    Inputs per core: srcw [CH, F], idx [128, SLOTS/16] i16,
    meta [128, D+2*B_TOTAL] f32.
    Output: part [TS, D, F] f32 (per-slot aggregates; host unpermutes).

    dense: None or dict(K=contract dim, FO=out feats) adding the Z epilogue:
      extra inputs aT [K, NS], bT [K, NS], wz [K, 2*FO], bz [1, FO]
      extra output z [NS, FO] with z = a^T rows @ wz[:, :FO] + b^T rows @
      wz[:, FO:] + bz  (per 128-row chunks).
    """
    nc = bacc.Bacc("TRN2", target_bir_lowering=False)
    srcw = nc.declare_dram_parameter("srcw", [CH, F], _DT, isOutput=False)
    idx = nc.declare_dram_parameter("idx", [P, SLOTS // 16], mybir.dt.int16, isOutput=False)
    meta = nc.declare_dram_parameter("meta", [P, D + 2 * B_TOTAL], _DT, isOutput=False)
    part = nc.declare_dram_parameter("part", [TS, D, F], _DT, isOutput=True)
    if dense is not None:
        KD, FO = dense["K"], dense["FO"]
        aT = nc.declare_dram_parameter("aT", [KD, NS], _DT, isOutput=False)
        bT = nc.declare_dram_parameter("bT", [KD, NS], _DT, isOutput=False)
        wz = nc.declare_dram_parameter("wz", [KD, 2 * FO], _DT, isOutput=False)
        bz = nc.declare_dram_parameter("bz", [1, FO], _DT, isOutput=False)
        z = nc.declare_dram_parameter("z", [NS, FO], _DT, isOutput=True)

    # slot -> tile boundaries
    tile_of_block = np.repeat(np.arange(len(NB)), NB)
    first_block = np.concatenate([[0], np.cumsum(NB)[:-1]])
    last_block = np.cumsum(NB) - 1

    with ExitStack() as ctx:
        tc = ctx.enter_context(tile.TileContext(nc))
        cpool = ctx.enter_context(tc.tile_pool(name="const", bufs=1))
        gpool = ctx.enter_context(tc.tile_pool(name="g", bufs=4))
        spool = ctx.enter_context(tc.tile_pool(name="s", bufs=8))
        opool = ctx.enter_context(tc.tile_pool(name="o", bufs=3))
        ppool = ctx.enter_context(tc.tile_pool(name="ps", bufs=4, space="PSUM"))

        idx_t = cpool.tile([P, SLOTS // 16], mybir.dt.int16)
        meta_t = cpool.tile([P, D + 2 * B_TOTAL], _DT)
        nc.sync.dma_start(out=idx_t[:], in_=idx[:])
        nc.sync.dma_start(out=meta_t[:], in_=meta[:])
        iota_f = meta_t[:, 0:D]

        psum = None
        cur_tile = -1
        for (b0, nb) in calls:
            g = gpool.tile([P, nb, F], _DT, tag="g")
            nc.gpsimd.dma_gather(
                g[:],
                srcw[:],
                idx_t[:, b0 * 8 : (b0 + nb) * 8],
                nb * P, nb * P, F,
            )
            for j in range(nb):
                blk = b0 + j
                t = int(tile_of_block[blk])
                S = spool.tile([P, D], _DT, tag="S")
                mc = D + 2 * blk
                nc.vector.tensor_scalar(
                    out=S[:], in0=iota_f,
                    scalar1=meta_t[:, mc : mc + 1],
                    scalar2=meta_t[:, mc + 1 : mc + 2],
                    op0=mybir.AluOpType.is_equal,
                    op1=mybir.AluOpType.mult,
                )
                if blk == first_block[t]:
                    psum = ppool.tile([D, F], _DT, space="PSUM", tag="acc")
                nc.tensor.matmul(
                    out=psum[:], lhsT=S[:], rhs=g[:, j, :],
                    start=(blk == first_block[t]),
                    stop=(blk == last_block[t]),
                )
                if blk == last_block[t]:
                    o = opool.tile([D, F], _DT, tag="o")
                    nc.vector.tensor_copy(o[:], psum[:])
                    nc.sync.dma_start(out=part[t], in_=o[:])

        if dense is not None:
            dpool = ctx.enter_context(tc.tile_pool(name="dz", bufs=3))
            zpool = ctx.enter_context(tc.tile_pool(name="zz", bufs=3))
            zps = ctx.enter_context(tc.tile_pool(name="zps", bufs=2, space="PSUM"))
            wz_t = cpool.tile([KD, 2 * FO], _DT)
            bz_t = cpool.tile([1, FO], _DT)
            ones_t = cpool.tile([1, P], _DT)
            nc.sync.dma_start(out=wz_t[:], in_=wz[:])
            nc.sync.dma_start(out=bz_t[:], in_=bz[:])
            nc.vector.memset(ones_t[:], 1.0)
            nchunks = -(-NS // P)
            for j in range(nchunks):
                m = min(P, NS - j * P)
                a_t = dpool.tile([KD, P], _DT, tag="a")
                b_t = dpool.tile([KD, P], _DT, tag="b")
                nc.sync.dma_start(out=a_t[:, :m], in_=aT[:, j * P : j * P + m])
                nc.sync.dma_start(out=b_t[:, :m], in_=bT[:, j * P : j * P + m])
                pz = zps.tile([P, FO], _DT, space="PSUM", tag="z")
                nc.tensor.matmul(out=pz[:m], lhsT=a_t[:, :m], rhs=wz_t[:, :FO],
                                 start=True, stop=False)
                nc.tensor.matmul(out=pz[:m], lhsT=b_t[:, :m], rhs=wz_t[:, FO:],
                                 start=False, stop=False)
                nc.tensor.matmul(out=pz[:m], lhsT=ones_t[:, :m], rhs=bz_t[:],
                                 start=False, stop=True)
                zo = zpool.tile([P, FO], _DT, tag="zo")
                nc.vector.tensor_copy(zo[:m], pz[:m])
                nc.sync.dma_start(out=z[j * P : j * P + m], in_=zo[:m])

    nc.compile()
    return nc


def _build_combine_program(F, FO, relu, scale2):
    """D launch: reduce 4 feature-major partials, combine with Z, matmul W.

    Inputs per core: zin [NS, FO], pT [4, F, NS], w [F, FO].
    Output: out [NS, FO] = act(zin + scale2 * (sum_i pT[i])^T @ w).
    """
    nc = bacc.Bacc("TRN2", target_bir_lowering=False)
    zin = nc.declare_dram_parameter("zin", [NS, FO], _DT, isOutput=False)
    pT = nc.declare_dram_parameter("pT", [4, F, NS], _DT, isOutput=False)
    w = nc.declare_dram_parameter("w", [F, FO], _DT, isOutput=False)
    out = nc.declare_dram_parameter("out", [NS, FO], _DT, isOutput=True)

    with ExitStack() as ctx:
        tc = ctx.enter_context(tile.TileContext(nc))
        cpool = ctx.enter_context(tc.tile_pool(name="const", bufs=1))
        dpool = ctx.enter_context(tc.tile_pool(name="d", bufs=3))
        rpool = ctx.enter_context(tc.tile_pool(name="r", bufs=3))
        opool = ctx.enter_context(tc.tile_pool(name="o", bufs=3))
        ppool = ctx.enter_context(tc.tile_pool(name="ps", bufs=2, space="PSUM"))

        w_t = cpool.tile([F, FO], _DT)
        nc.sync.dma_start(out=w_t[:], in_=w[:])

        nchunks = -(-NS // P)
        for j in range(nchunks):
            m = min(P, NS - j * P)
            pt = dpool.tile([F, 4, P], _DT, tag="pt")
            nc.sync.dma_start(out=pt[:, :, :m], in_=pT[:, :, j * P : j * P + m].rearrange("a b c -> b a c"))
            red = rpool.tile([F, P], _DT, tag="red")
            nc.vector.tensor_add(out=red[:, :m], in0=pt[:, 0, :m], in1=pt[:, 1, :m])
            nc.vector.tensor_add(out=red[:, :m], in0=red[:, :m], in1=pt[:, 2, :m])
            nc.vector.tensor_add(out=red[:, :m], in0=red[:, :m], in1=pt[:, 3, :m])
            pz = ppool.tile([P, FO], _DT, space="PSUM", tag="z")
            nc.tensor.matmul(out=pz[:m], lhsT=red[:, :m], rhs=w_t[:],
                             start=True, stop=True)
            zt = dpool.tile([P, FO], _DT, tag="zt")
            nc.sync.dma_start(out=zt[:m], in_=zin[j * P : j * P + m])
            oo = opool.tile([P, FO], _DT, tag="oo")
            # oo = zin + scale2 * psum
            nc.vector.tensor_scalar(
                out=oo[:m], in0=pz[:m],
                scalar1=float(scale2), scalar2=None,
                op0=mybir.AluOpType.mult,
            )
            nc.vector.tensor_add(out=oo[:m], in0=oo[:m], in1=zt[:m])
            if relu:
                nc.scalar.activation(oo[:m], oo[:m], mybir.ActivationFunctionType.Relu)
            nc.sync.dma_start(out=out[j * P : j * P + m], in_=oo[:m])

    nc.compile()
    return nc


# ---------------------------------------------------------------------------
# host glue
# ---------------------------------------------------------------------------

def _pad_rows(a, rows):
    out = np.zeros((rows, a.shape[1]), np.float32)
    out[: a.shape[0]] = a
    return out


def _reduce_partials(parts, tile_ids):
    """parts: list of 8 arrays [TS, D, F] in per-core slot order.
    Returns full [NPAD, F] (sum of the 4 chunk-partials per half)."""
    F = parts[0].shape[2]
    full = np.zeros((NPAD, F), np.float32)
    for c in range(NCORES):
        half = c // NCHUNK
        un = np.zeros((TS, D, F), np.float32)
        un[tile_ids[c]] = parts[c]
        full[half * HALF : (half + 1) * HALF] += un.reshape(HALF, F)
    return full


def _run(nc, in_maps):
    res = run_bass_kernel_spmd(nc, in_maps, list(range(NCORES)))
    return res.results


class _Programs:
    """Compiled program cache for one graph structure."""

    def __init__(self, g):
        self.g = g
        NB, calls, BT, SL = g["NB"], g["calls"], g["B_TOTAL"], g["SLOTS"]
        self.pA = _build_pass_program(F_IN, NB, calls, BT, SL,
                                      dense=dict(K=F_IN, FO=F_HID))
        self.pB = _build_pass_program(F_HID, NB, calls, BT, SL,
                                      dense=dict(K=F_HID, FO=F_OUT))
        self.d1 = _build_combine_program(F_IN, F_HID, relu=True, scale2=2.0)
        self.d2 = _build_combine_program(F_HID, F_OUT, relu=False, scale2=2.0)


def _pass_inputs(g, src_full, zin_a=None, zin_b=None, wz=None, bzv=None, F=None, KD=None, FO=None):
    """Build per-core in_maps for a pass program."""
    maps = []
    for c in range(NCORES):
        chunk = c % NCHUNK
        m = {
            "srcw": src_full[chunk * CH : (chunk + 1) * CH],
            "idx": g["idxw"][c],
            "meta": g["meta"][c],
        }
        if wz is not None:
            sl = slice(c * NS, (c + 1) * NS)
            m["aT"] = np.ascontiguousarray(zin_a[sl].T) if zin_a is not None else np.zeros((KD, NS), np.float32)
            m["bT"] = np.ascontiguousarray(zin_b[sl].T) if zin_b is not None else np.zeros((KD, NS), np.float32)
            m["wz"] = wz
            m["bz"] = bzv.reshape(1, -1)
        maps.append(m)
    return maps


def _combine_inputs(zs, full_parts_T, w):
    """zs: [NPAD, FO] z outputs per node; full_parts_T: [8 cores][4, F, NS]."""
    maps = []
    for c in range(NCORES):
        sl = slice(c * NS, (c + 1) * NS)
        maps.append({
            "zin": zs[sl],
            "pT": full_parts_T[c],
            "w": w,
        })
    return maps


def _partials_T_for_cores(parts, tile_ids, F):
    """Rearrange 8 per-core slot partials into per-node-slice stacked
    feature-major partial tensors for the combine launch.

    Returns list of 8 arrays [4, F, NS]: for node-slice core c, the 4 chunk
    partials restricted to its rows, transposed."""
    # unpermute each core's partial to [NPAD? half rows, F]
    un = np.zeros((NCORES, HALF, F), np.float32)
    for c in range(NCORES):
        tmp = np.zeros((TS, D, F), np.float32)
        tmp[tile_ids[c]] = parts[c]
        un[c] = tmp.reshape(HALF, F)
    out = []
    for c in range(NCORES):
        r0 = c * NS
        arr = np.zeros((4, F, NS), np.float32)
        for i in range(4):
            # partial for rows [r0, r0+NS) lives in core (half*4 + i)
            half = r0 // HALF
            assert (r0 + NS - 1) // HALF == half
            src = un[half * NCHUNK + i]
            arr[i] = src[r0 - half * HALF : r0 - half * HALF + NS].T
        out.append(np.ascontiguousarray(arr))
    return out


def kernel(x, edge_index, edge_weight, W1, b1, W2, b2):
    x = np.asarray(x, np.float32)
    edge_index = np.asarray(edge_index)
    edge_weight = np.asarray(edge_weight, np.float32)
    W1 = np.asarray(W1, np.float32)
    b1 = np.asarray(b1, np.float32)
    W2 = np.asarray(W2, np.float32)
    b2 = np.asarray(b2, np.float32)

    g = _prep_graph(edge_index, edge_weight)
    progs = _Programs(g)

    xpad = _pad_rows(x, NPAD)

    # dense weight combos
    w1z = np.concatenate([W1[0] - W1[2], W1[1]], axis=1)  # [128, 128]
    w2z = np.concatenate([W2[0] - W2[2], W2[1]], axis=1)  # [64, 80]

    # P1: Tx1 partials (z inputs zero, z output ignored)
    maps = _pass_inputs(g, xpad, None, None, w1z, b1, F=F_IN, KD=F_IN, FO=F_HID)
    res = _run(progs.pA, maps)
    tx1 = _reduce_partials([r["part"] for r in res], g["tile_ids"])

    # P2: Tx2 partials + Z1 = x(W10-W12) + Tx1 W11 + b1
    maps = _pass_inputs(g, tx1, xpad, tx1, w1z, b1, F=F_IN, KD=F_IN, FO=F_HID)
    res = _run(progs.pA, maps)
    tx2_parts = [r["part"] for r in res]
    z1 = np.concatenate([r["z"] for r in res], axis=0)  # [NPAD, F_HID]

    # D1: h = relu(Z1 + 2 * Tx2 * W12)
    pT = _partials_T_for_cores(tx2_parts, g["tile_ids"], F_IN)
    maps = _combine_inputs(z1, pT, W1[2])
    res = _run(progs.d1, maps)
    h = np.concatenate([r["out"] for r in res], axis=0)  # [NPAD, F_HID]

    # P3: U1 partials
    maps = _pass_inputs(g, h, None, None, w2z, b2, F=F_HID, KD=F_HID, FO=F_OUT)
    res = _run(progs.pB, maps)
    u1 = _reduce_partials([r["part"] for r in res], g["tile_ids"])

    # P4: U2 partials + Z2 = h(W20-W22) + U1 W21 + b2
    maps = _pass_inputs(g, u1, h, u1, w2z, b2, F=F_HID, KD=F_HID, FO=F_OUT)
    res = _run(progs.pB, maps)
    u2_parts = [r["part"] for r in res]
    z2 = np.concatenate([r["z"] for r in res], axis=0)

    # D2: out = Z2 + 2 * U2 * W22
    pT = _partials_T_for_cores(u2_parts, g["tile_ids"], F_HID)
    maps = _combine_inputs(z2, pT, W2[2])
    res = _run(progs.d2, maps)
    out = np.concatenate([r["out"] for r in res], axis=0)

    return out[:N]


# revision 14
# speedup vs baseline: 2.0724x; 2.0724x over previous
"""ChebNet (2-layer ChebConv, K=3) on 8 Trainium2 NeuronCores.

Strategy
--------
Math: propagation commutes with the per-order weight matmul, so the two
ChebConv layers reduce to 4 sparse propagations on raw features plus tiny
dense matmuls:
    L1: out1 = x(W10-W12) + Tx1*W11 + 2*Tx2*W12 + b1,  Tx1 = L x, Tx2 = L Tx1
    h = relu(out1)
    L2: out2 = h(W20-W22) + U1*W21 + 2*U2*W22 + b2,    U1 = L h, U2 = L U1
where L[c,r] = sum over edges (r->c) of -dinv[r]*w*dinv[c]  (PyG ChebConv
normalization with lambda_max=2).

Sharding: each of the 8 cores owns edges with source in one of 4 contiguous
25024-row windows and dest in one of 2 halves (4 chunks x 2 halves). Each
core gathers source rows from its window (int16 dma_gather indices), forms
per-edge messages, and aggregates them into its half's destination tiles
with an is_equal-selector matmul on the tensor engine. Host reduces the 4
partial aggregates per half between launches (pure data movement + adds).

Device pipeline per pass: dma_gather (1024 rows/call) -> DVE builds
S[t,d] = norm[t] * (iota[d] == local_dest[t]) -> PE matmul accumulates
psum[d, :] += S^T @ messages -> psum copied out per 64-node dest tile.
"""
import numpy as np
from contextlib import ExitStack

import concourse.bass as bass
import concourse.bacc as bacc
import concourse.mybir as mybir
import concourse.tile as tile
from concourse.bass_utils import run_bass_kernel_spmd

# problem constants (hardcoded per harness contract)
N = 100000
E = 1600000
F_IN = 128
F_HID = 64
F_OUT = 40
K = 3

P = 128
D = 64                 # dest-tile width (nodes per psum tile)
NPAD = 100096          # padded node count: /128 = 782, /64 = 1564
NCHUNK = 4
CH = NPAD // NCHUNK    # 25024 source rows per chunk (< 32768 for int16 idx)
NHALF = 2
HALF = NPAD // NHALF   # 50048 dest rows per half
TS = HALF // D         # 782 dest tiles per half
NCORES = 8
NS = NPAD // NCORES    # 12512 nodes per core for dense epilogues
CALL_BLOCKS = 8        # 1024 gather rows per dma_gather (descriptor ring limit)

_DT = mybir.dt.float32


# ---------------------------------------------------------------------------
# host-side graph preprocessing
# ---------------------------------------------------------------------------

def _prep_graph(edge_index, edge_weight):
    """Partition edges into 8 (chunk, half) cores; build per-core slot arrays.

    Returns dict with per-core int16 gather indices (wrapped layout), meta
    (ld/norm) arrays, the shared block structure NB[slot->nblocks], call plan,
    and per-core tile-id permutations.
    """
    row = np.ascontiguousarray(edge_index[0]).astype(np.int64)
    col = np.ascontiguousarray(edge_index[1]).astype(np.int64)
    w = np.ascontiguousarray(edge_weight).astype(np.float32)

    deg = np.bincount(row, weights=w.astype(np.float64), minlength=N).astype(np.float32)
    dinv = np.where(deg > 0, 1.0 / np.sqrt(np.maximum(deg, 1e-30)), 0.0).astype(np.float32)
    norm = (-dinv[row] * w * dinv[col]).astype(np.float32)

    chunk = row // CH
    half = col // HALF
    core_of_edge = (half * NCHUNK + chunk).astype(np.int64)

    # per (core, tile) edge counts; tile id local to the half
    ltile = (col % HALF) // D
    ld = (col % HALF) % D

    cores = []
    nblocks_sorted = []
    for c in range(NCORES):
        sel = np.nonzero(core_of_edge == c)[0]
        # order edges by local tile for contiguous tile runs
        order = np.argsort(ltile[sel], kind="stable")
        sel = sel[order]
        t_of_e = ltile[sel]
        counts = np.bincount(t_of_e, minlength=TS)
        nb = np.maximum(1, -(-counts // P))  # ceil, min 1 block per tile
        # sort tiles by descending block count, stable by tile id
        perm = np.lexsort((np.arange(TS), -nb))
        cores.append(dict(sel=sel, counts=counts, nb=nb, perm=perm))
        nblocks_sorted.append(nb[perm])

    NB = np.max(np.stack(nblocks_sorted), axis=0)  # shared per-slot block count
    B_TOTAL = int(NB.sum())
    SLOTS = B_TOTAL * P

    # call plan: runs of <= CALL_BLOCKS blocks (identical for all cores)
    calls = []
    b = 0
    while b < B_TOTAL:
        n = min(CALL_BLOCKS, B_TOTAL - b)
        calls.append((b, n))
        b += n

    # per-core slot arrays; pad slots gather row 0 (zeroed by ld=-1 in S)
    idx16 = np.zeros((NCORES, SLOTS), np.int16)
    ld_f = np.full((NCORES, SLOTS), -1.0, np.float32)
    nrm_f = np.zeros((NCORES, SLOTS), np.float32)
    tile_ids = np.zeros((NCORES, TS), np.int64)

    slot_tile_start = np.concatenate([[0], np.cumsum(NB)]) * P  # per sorted slot
    for c in range(NCORES):
        st = cores[c]
        sel, counts, perm = st["sel"], st["counts"], st["perm"]
        tile_ids[c] = perm
        # edge offsets per tile in the tile-ordered edge list
        e_start = np.concatenate([[0], np.cumsum(counts)])
        ch_base = (c % NCHUNK) * CH
        for s in range(TS):
            t = perm[s]
            cnt = counts[t]
            if cnt == 0:
                continue
            eids = sel[e_start[t] : e_start[t] + cnt]
            base = slot_tile_start[s]
            idx16[c, base : base + cnt] = (row[eids] - ch_base).astype(np.int16)
            ld_f[c, base : base + cnt] = ld[eids].astype(np.float32)
            nrm_f[c, base : base + cnt] = norm[eids]

    # wrap idx to dma_gather layout [16, SLOTS/16] at [i%16, i//16], tile 8x
    ii = np.arange(SLOTS)
    idxw = np.zeros((NCORES, 16, SLOTS // 16), np.int16)
    idxw[:, ii % 16, ii // 16] = idx16
    idxw = np.tile(idxw, (1, 8, 1))  # [NCORES, 128, SLOTS/16]

    # meta [128, D + 2*B_TOTAL]: iota then per-block (ld, norm) column pairs.
    # slot i of call k maps to partition i%128, block (global) i//128.
    meta = np.zeros((NCORES, P, D + 2 * B_TOTAL), np.float32)
    meta[:, :, :D] = np.arange(D, dtype=np.float32)[None, None, :]
    ld_pb = ld_f.reshape(NCORES, B_TOTAL, P).transpose(0, 2, 1)   # [C, 128, B]
    nrm_pb = nrm_f.reshape(NCORES, B_TOTAL, P).transpose(0, 2, 1)
    meta[:, :, D::2] = ld_pb
    meta[:, :, D + 1 :: 2] = nrm_pb

    return dict(
        NB=NB, B_TOTAL=B_TOTAL, SLOTS=SLOTS, calls=calls,
        idxw=idxw, meta=meta, tile_ids=tile_ids, dinv=dinv,
    )


# ---------------------------------------------------------------------------
# device program builders
# ---------------------------------------------------------------------------

def _build_pass_program(F, NB, calls, B_TOTAL, SLOTS, dense=None):
    """One propagation pass: gather + selector-matmul aggregation.

    Inputs per core: srcw [CH, F], idx [128, SLOTS/16] i16,
    meta [128, D+2*B_TOTAL] f32.
    Output: part [TS, D, F] f32 (per-slot aggregates; host unpermutes).

    dense: None or dict(K=contract dim, FO=out feats) adding the Z epilogue:
      extra inputs aT [K, NS], bT [K, NS], wz [K, 2*FO], bz [1, FO]
      extra output z [NS, FO] with z = a^T rows @ wz[:, :FO] + b^T rows @
      wz[:, FO:] + bz  (per 128-row chunks).
    """
    nc = bacc.Bacc("TRN2", target_bir_lowering=False)
    srcw = nc.declare_dram_parameter("srcw", [CH, F], _DT, isOutput=False)
    idx = nc.declare_dram_parameter("idx", [P, SLOTS // 16], mybir.dt.int16, isOutput=False)
    meta = nc.declare_dram_parameter("meta", [P, D + 2 * B_TOTAL], _DT, isOutput=False)
    # part laid out [D, TS, F] so grouped tile writes are contiguous 4KB runs
    part = nc.declare_dram_parameter("part", [D, TS, F], _DT, isOutput=True)
    if dense is not None:
        KD, FO = dense["K"], dense["FO"]
        aT = nc.declare_dram_parameter("aT", [KD, NS], _DT, isOutput=False)
        bT = nc.declare_dram_parameter("bT", [KD, NS], _DT, isOutput=False)
        wz = nc.declare_dram_parameter("wz", [KD, 2 * FO], _DT, isOutput=False)
        bz = nc.declare_dram_parameter("bz", [1, FO], _DT, isOutput=False)
        z = nc.declare_dram_parameter("z", [NS, FO], _DT, isOutput=True)

    # slot -> tile boundaries
    tile_of_block = np.repeat(np.arange(len(NB)), NB)
    first_block = np.concatenate([[0], np.cumsum(NB)[:-1]])
    last_block = np.cumsum(NB) - 1
    OG = 8  # output tiles per grouped DRAM write

    with ExitStack() as ctx:
        tc = ctx.enter_context(tile.TileContext(nc))
        cpool = ctx.enter_context(tc.tile_pool(name="const", bufs=1))
        gpool = ctx.enter_context(tc.tile_pool(name="g", bufs=4))
        spool = ctx.enter_context(tc.tile_pool(name="s", bufs=8))
        opool = ctx.enter_context(tc.tile_pool(name="o", bufs=3))
        ppool = ctx.enter_context(tc.tile_pool(name="ps", bufs=4, space="PSUM"))

        idx_t = cpool.tile([P, SLOTS // 16], mybir.dt.int16)
        meta_t = cpool.tile([P, D + 2 * B_TOTAL], _DT)
        nc.sync.dma_start(out=idx_t[:], in_=idx[:])
        nc.sync.dma_start(out=meta_t[:], in_=meta[:])
        iota_f = meta_t[:, 0:D]

        psum = None
        cur_tile = -1
        for (b0, nb) in calls:
            g = gpool.tile([P, nb, F], _DT, tag="g")
            nc.gpsimd.dma_gather(
                g[:],
                srcw[:],
                idx_t[:, b0 * 8 : (b0 + nb) * 8],
                nb * P, nb * P, F,
            )
            for j in range(nb):
                blk = b0 + j
                t = int(tile_of_block[blk])
                S = spool.tile([P, D], _DT, tag="S")
                mc = D + 2 * blk
                nc.vector.tensor_scalar(
                    out=S[:], in0=iota_f,
                    scalar1=meta_t[:, mc : mc + 1],
                    scalar2=meta_t[:, mc + 1 : mc + 2],
                    op0=mybir.AluOpType.is_equal,
                    op1=mybir.AluOpType.mult,
                )
                if blk == first_block[t]:
                    psum = ppool.tile([D, F], _DT, space="PSUM", tag="acc")
                nc.tensor.matmul(
                    out=psum[:], lhsT=S[:], rhs=g[:, j, :],
                    start=(blk == first_block[t]),
                    stop=(blk == last_block[t]),
                )
                if blk == last_block[t]:
                    gi, go = t // OG, t % OG
                    if go == 0:
                        og = opool.tile([D, OG, F], _DT, tag="o")
                    nc.vector.tensor_copy(og[:, go, :], psum[:])
                    if go == OG - 1 or t == len(NB) - 1:
                        nc.sync.dma_start(
                            out=part[:, gi * OG : gi * OG + go + 1, :],
                            in_=og[:, : go + 1, :],
                        )

        if dense is not None:
            dpool = ctx.enter_context(tc.tile_pool(name="dz", bufs=3))
            zpool = ctx.enter_context(tc.tile_pool(name="zz", bufs=3))
            zps = ctx.enter_context(tc.tile_pool(name="zps", bufs=2, space="PSUM"))
            wz_t = cpool.tile([KD, 2 * FO], _DT)
            bz_t = cpool.tile([1, FO], _DT)
            ones_t = cpool.tile([1, P], _DT)
            nc.sync.dma_start(out=wz_t[:], in_=wz[:])
            nc.sync.dma_start(out=bz_t[:], in_=bz[:])
            nc.vector.memset(ones_t[:], 1.0)
            SC = 1024  # super-chunk columns per load
            nsup = -(-NS // SC)
            for sj in range(nsup):
                sc = min(SC, NS - sj * SC)
                a_t = dpool.tile([KD, SC], _DT, tag="a")
                b_t = dpool.tile([KD, SC], _DT, tag="b")
                nc.sync.dma_start(out=a_t[:, :sc], in_=aT[:, sj * SC : sj * SC + sc])
                nc.sync.dma_start(out=b_t[:, :sc], in_=bT[:, sj * SC : sj * SC + sc])
                zo = zpool.tile([P, SC // P, FO], _DT, tag="zo")
                nj = -(-sc // P)
                for j in range(nj):
                    m = min(P, sc - j * P)
                    pz = zps.tile([P, FO], _DT, space="PSUM", tag="z")
                    nc.tensor.matmul(out=pz[:m], lhsT=a_t[:, j * P : j * P + m],
                                     rhs=wz_t[:, :FO], start=True, stop=False)
                    nc.tensor.matmul(out=pz[:m], lhsT=b_t[:, j * P : j * P + m],
                                     rhs=wz_t[:, FO:], start=False, stop=False)
                    nc.tensor.matmul(out=pz[:m], lhsT=ones_t[:, :m], rhs=bz_t[:],
                                     start=False, stop=True)
                    nc.vector.tensor_copy(zo[:m, j, :], pz[:m])
                if sc == SC:
                    nc.sync.dma_start(
                        out=z[sj * SC : sj * SC + sc].rearrange("(j p) f -> p j f", p=P),
                        in_=zo[:, :nj, :],
                    )
                else:
                    for j in range(nj):
                        m = min(P, sc - j * P)
                        nc.sync.dma_start(
                            out=z[sj * SC + j * P : sj * SC + j * P + m],
                            in_=zo[:m, j, :],
                        )

    nc.compile()
    return nc


def _build_combine_program(F, FO, relu, scale2):
    """D launch: combine Z with the reduced propagation result, matmul W.

    Inputs per core: zin [NS, FO], pT [F, NS] (host-reduced, feature-major),
    w [F, FO].  Output: out [NS, FO] = act(zin + scale2 * pT^T @ w).
    """
    nc = bacc.Bacc("TRN2", target_bir_lowering=False)
    zin = nc.declare_dram_parameter("zin", [NS, FO], _DT, isOutput=False)
    pT = nc.declare_dram_parameter("pT", [F, NS], _DT, isOutput=False)
    w = nc.declare_dram_parameter("w", [F, FO], _DT, isOutput=False)
    out = nc.declare_dram_parameter("out", [NS, FO], _DT, isOutput=True)

    with ExitStack() as ctx:
        tc = ctx.enter_context(tile.TileContext(nc))
        cpool = ctx.enter_context(tc.tile_pool(name="const", bufs=1))
        dpool = ctx.enter_context(tc.tile_pool(name="d", bufs=3))
        opool = ctx.enter_context(tc.tile_pool(name="o", bufs=3))
        ppool = ctx.enter_context(tc.tile_pool(name="ps", bufs=4, space="PSUM"))

        w_t = cpool.tile([F, FO], _DT)
        nc.sync.dma_start(out=w_t[:], in_=w[:])

        SC = 1024
        nsup = -(-NS // SC)
        for sj in range(nsup):
            sc = min(SC, NS - sj * SC)
            nj = -(-sc // P)
            pt = dpool.tile([F, SC], _DT, tag="pt")
            zt = dpool.tile([P, SC // P, FO], _DT, tag="zt")
            nc.sync.dma_start(out=pt[:, :sc], in_=pT[:, sj * SC : sj * SC + sc])
            if sc == SC:
                nc.sync.dma_start(
                    out=zt[:],
                    in_=zin[sj * SC : (sj + 1) * SC].rearrange("(j p) f -> p j f", p=P),
                )
            else:
                for j in range(nj):
                    m = min(P, sc - j * P)
                    nc.sync.dma_start(out=zt[:m, j, :],
                                      in_=zin[sj * SC + j * P : sj * SC + j * P + m])
            oo = opool.tile([P, SC // P, FO], _DT, tag="oo")
            for j in range(nj):
                m = min(P, sc - j * P)
                pz = ppool.tile([P, FO], _DT, space="PSUM", tag="z")
                nc.tensor.matmul(out=pz[:m], lhsT=pt[:, j * P : j * P + m],
                                 rhs=w_t[:], start=True, stop=True)
                # oo = zin + scale2 * psum
                nc.vector.tensor_scalar(
                    out=oo[:m, j, :], in0=pz[:m],
                    scalar1=float(scale2), scalar2=None,
                    op0=mybir.AluOpType.mult,
                )
                nc.vector.tensor_add(out=oo[:m, j, :], in0=oo[:m, j, :], in1=zt[:m, j, :])
                if relu:
                    nc.scalar.activation(oo[:m, j, :], oo[:m, j, :],
                                         mybir.ActivationFunctionType.Relu)
            if sc == SC:
                nc.sync.dma_start(
                    out=out[sj * SC : (sj + 1) * SC].rearrange("(j p) f -> p j f", p=P),
                    in_=oo[:],
                )
            else:
                for j in range(nj):
                    m = min(P, sc - j * P)
                    nc.sync.dma_start(out=out[sj * SC + j * P : sj * SC + j * P + m],
                                      in_=oo[:m, j, :])

    nc.compile()
    return nc


# ---------------------------------------------------------------------------
# host glue
# ---------------------------------------------------------------------------

def _pad_rows(a, rows):
    out = np.zeros((rows, a.shape[1]), np.float32)
    out[: a.shape[0]] = a
    return out


def _reduce_partials(parts, tile_ids):
    """parts: list of 8 arrays [D, TS, F] in per-core slot order.
    Returns full [NPAD, F] (sum of the 4 chunk-partials per half)."""
    F = parts[0].shape[2]
    full = np.zeros((NPAD, F), np.float32)
    for c in range(NCORES):
        half = c // NCHUNK
        un = np.zeros((TS, D, F), np.float32)
        un[tile_ids[c]] = parts[c].transpose(1, 0, 2)
        full[half * HALF : (half + 1) * HALF] += un.reshape(HALF, F)
    return full


def _run(nc, in_maps):
    res = run_bass_kernel_spmd(nc, in_maps, list(range(NCORES)))
    return res.results


class _Programs:
    """Compiled program cache for one graph structure."""

    def __init__(self, g):
        self.g = g
        NB, calls, BT, SL = g["NB"], g["calls"], g["B_TOTAL"], g["SLOTS"]
        self.pA = _build_pass_program(F_IN, NB, calls, BT, SL,
                                      dense=dict(K=F_IN, FO=F_HID))
        self.pB = _build_pass_program(F_HID, NB, calls, BT, SL,
                                      dense=dict(K=F_HID, FO=F_OUT))
        self.d1 = _build_combine_program(F_IN, F_HID, relu=True, scale2=2.0)
        self.d2 = _build_combine_program(F_HID, F_OUT, relu=False, scale2=2.0)


def _pass_inputs(g, src_full, zin_a=None, zin_b=None, wz=None, bzv=None, F=None, KD=None, FO=None):
    """Build per-core in_maps for a pass program."""
    maps = []
    for c in range(NCORES):
        chunk = c % NCHUNK
        m = {
            "srcw": src_full[chunk * CH : (chunk + 1) * CH],
            "idx": g["idxw"][c],
            "meta": g["meta"][c],
        }
        if wz is not None:
            sl = slice(c * NS, (c + 1) * NS)
            m["aT"] = np.ascontiguousarray(zin_a[sl].T) if zin_a is not None else np.zeros((KD, NS), np.float32)
            m["bT"] = np.ascontiguousarray(zin_b[sl].T) if zin_b is not None else np.zeros((KD, NS), np.float32)
            m["wz"] = wz
            m["bz"] = bzv.reshape(1, -1)
        maps.append(m)
    return maps


def _combine_inputs(zs, reduced_full, w):
    """zs: [NPAD, FO] z rows; reduced_full: [NPAD, F] reduced propagation."""
    maps = []
    for c in range(NCORES):
        sl = slice(c * NS, (c + 1) * NS)
        maps.append({
            "zin": zs[sl],
            "pT": np.ascontiguousarray(reduced_full[sl].T),
            "w": w,
        })
    return maps


def kernel(x, edge_index, edge_weight, W1, b1, W2, b2):
    x = np.asarray(x, np.float32)
    edge_index = np.asarray(edge_index)
    edge_weight = np.asarray(edge_weight, np.float32)
    W1 = np.asarray(W1, np.float32)
    b1 = np.asarray(b1, np.float32)
    W2 = np.asarray(W2, np.float32)
    b2 = np.asarray(b2, np.float32)

    g = _prep_graph(edge_index, edge_weight)
    progs = _Programs(g)

    xpad = _pad_rows(x, NPAD)

    # dense weight combos
    w1z = np.concatenate([W1[0] - W1[2], W1[1]], axis=1)  # [128, 128]
    w2z = np.concatenate([W2[0] - W2[2], W2[1]], axis=1)  # [64, 80]

    # P1: Tx1 partials (z inputs zero, z output ignored)
    maps = _pass_inputs(g, xpad, None, None, w1z, b1, F=F_IN, KD=F_IN, FO=F_HID)
    res = _run(progs.pA, maps)
    tx1 = _reduce_partials([r["part"] for r in res], g["tile_ids"])

    # P2: Tx2 partials + Z1 = x(W10-W12) + Tx1 W11 + b1
    maps = _pass_inputs(g, tx1, xpad, tx1, w1z, b1, F=F_IN, KD=F_IN, FO=F_HID)
    res = _run(progs.pA, maps)
    tx2 = _reduce_partials([r["part"] for r in res], g["tile_ids"])
    z1 = np.concatenate([r["z"] for r in res], axis=0)  # [NPAD, F_HID]

    # D1: h = relu(Z1 + 2 * Tx2 * W12)
    maps = _combine_inputs(z1, tx2, W1[2])
    res = _run(progs.d1, maps)
    h = np.concatenate([r["out"] for r in res], axis=0)  # [NPAD, F_HID]

    # P3: U1 partials
    maps = _pass_inputs(g, h, None, None, w2z, b2, F=F_HID, KD=F_HID, FO=F_OUT)
    res = _run(progs.pB, maps)
    u1 = _reduce_partials([r["part"] for r in res], g["tile_ids"])

    # P4: U2 partials + Z2 = h(W20-W22) + U1 W21 + b2
    maps = _pass_inputs(g, u1, h, u1, w2z, b2, F=F_HID, KD=F_HID, FO=F_OUT)
    res = _run(progs.pB, maps)
    u2 = _reduce_partials([r["part"] for r in res], g["tile_ids"])
    z2 = np.concatenate([r["z"] for r in res], axis=0)

    # D2: out = Z2 + 2 * U2 * W22
    maps = _combine_inputs(z2, u2, W2[2])
    res = _run(progs.d2, maps)
    out = np.concatenate([r["out"] for r in res], axis=0)

    return out[:N]
